# revision 2
# baseline (speedup 1.0000x reference)
"""MoE (top-2 of 8 experts) SwiGLU FFN on 8 Trainium2 NeuronCores.

Strategy (expert-parallel, per the sharding hint):
  - Router (x @ w_gate -> softmax -> top-2) computed host-side on jax-CPU with
    the exact ops the reference uses, so expert selection matches the
    reference bit-for-bit. This is the "dispatch tokens by topk_idx" step.
  - Core e receives only the tokens routed to expert e (gathered, transposed,
    and pre-cast to bf16 host-side), plus expert e's weights pre-packed into
    the SBUF tile layout (so every device DMA is a single contiguous 2D
    HWDGE transfer). All cores run one SPMD program sized to
    cap = max tokens per expert (zero-padded).
  - Device computes y_e^T = wo_e^T @ (silu(wg_e^T x^T) * (wi_e^T x^T)) with
    bf16 matmuls accumulating in fp32 PSUM. Tokens stay on the PSUM free
    dimension throughout, so no on-device transposes are needed: lhsT
    operands are the natural wi/wg [C,H] and wo [H,C] layouts.
  - Host combines: out[t] = val0[t]*y_{e0}[t] + val1[t]*y_{e1}[t].

Perf structure (measured on HW, slope-timed):
  - stage-1 u/g accumulation chains are interleaved matmul-by-matmul so
    consecutive matmuls never hit the same PSUM bank back-to-back
    (sequential chains cost ~+70ns/MM extra).
  - PSUM split: 3 bufs psu + 3 psg (stage 1) + 2 psy (stage 2) = 8 banks.
  - 5 even token tiles (widths cap/5 rounded to mult-of-4, all <= 512)
    instead of 4x512 + ragged tail.
  - yt output in bf16 (halves PSUM evacuation + output DMA).
  This measures ~357us/rep vs ~427us for the previous layout in the same
  session, within 2% of the bf16 PE streaming floor at cap=2184.
"""

import numpy as np
import ml_dtypes

import concourse.bass as bass
import concourse.mybir as mybir
import concourse.tile as tile
from concourse.bass_utils import run_bass_kernel_spmd

N_CORES = 8
N_EXPERTS = 8
TOP_K = 2
B, T, C, H = 4, 2048, 1024, 2048
CC = C // 128           # contraction chunks over C
HH = H // 128           # chunks over H
NTILES = 5              # even token tiles per core
HBW = 512               # stage-1 weight block width (columns of H)
CBW = 512               # stage-2 weight block width (columns of C)
HB = H // HBW
CB = C // CBW
BF16 = mybir.dt.bfloat16
F32 = mybir.dt.float32


def _split_multi_waits(nc, max_waits=1):
    """This walrus build rejects >1 sync-wait per instruction. Peel extra
    waits onto single-wait EventSemaphore instructions inserted just before,
    on the same engine (identical blocking semantics)."""
    n_split = 0
    for fn in nc.m.functions:
        for bb in fn.blocks:
            out = []
            changed = False
            for inst in bb.instructions:
                si = inst.sync_info
                waits = list(si.on_wait) if si is not None else []
                if len(waits) > max_waits:
                    head, keep = waits[:-max_waits], waits[-max_waits:]
                    for j, w in enumerate(head):
                        out.append(mybir.InstEventSemaphore(
                            name=f"{inst.name}-wspl{j}",
                            engine=inst.engine,
                            sync_info=mybir.SyncInfo(on_wait=[w], on_update=[]),
                        ))
                    inst.sync_info = mybir.SyncInfo(
                        on_wait=keep, on_update=list(si.on_update))
                    changed = True
                    n_split += 1
                out.append(inst)
            if changed:
                bb.instructions = out
    return n_split


def tok_tiles_for(cap, ntiles=NTILES):
    """Even split of cap into ntiles widths, each a multiple of 4."""
    assert cap % 4 == 0
    q = cap // 4
    base = q // ntiles
    rem = q - base * ntiles
    widths = [4 * (base + (1 if i < rem else 0)) for i in range(ntiles)]
    tiles = []
    t0 = 0
    for w in widths:
        tiles.append((t0, w))
        t0 += w
    return tiles


def build_program(cap, reps=1):
    """One SPMD program: expert FFN over [cap] tokens (token dim = PSUM free
    dim everywhere). reps>1 repeats the whole compute (timing only).

    DRAM inputs are already in SBUF tile layout, bf16:
      xtb [128, CC*cap]     token-tile-major per tok_tiles_for(cap):
                            xtb[p, off(ti)*CC + cc*tw + t] =
                            x^T[cc*128+p, t0(ti)+t]
      wib [128, HB*CC*HBW]  wib[p, (hb*CC+cc)*HBW+f] = wi[cc*128+p, hb*HBW+f]
      wgb [128, HB*CC*HBW]  same layout as wib
      wob [128, CB*HH*CBW]  wob[p, (cb*HH+hh)*CBW+f] = wo[hh*128+p, cb*CBW+f]
    Output yt [C, cap] bf16 (y^T, one row block per c-chunk).
    """
    tiles = tok_tiles_for(cap)
    assert all(w <= 512 for _, w in tiles)
    BLK = CC * HBW
    BLK2 = HH * CBW

    nc = bass.Bass()
    xtb = nc.dram_tensor("xtb", [128, CC * cap], BF16, kind="ExternalInput")
    wib = nc.dram_tensor("wib", [128, HB * CC * HBW], BF16, kind="ExternalInput")
    wgb = nc.dram_tensor("wgb", [128, HB * CC * HBW], BF16, kind="ExternalInput")
    wob = nc.dram_tensor("wob", [128, CB * HH * CBW], BF16, kind="ExternalInput")
    yt = nc.dram_tensor("yt", [C, cap], BF16, kind="ExternalOutput")
    # tiny output: fetching it waits for program completion without paying
    # the full yt transfer through the tunnel (timing use)
    done = nc.dram_tensor("done", [1, 8], BF16, kind="ExternalOutput")

    with tile.TileContext(nc) as tc:
        with tc.tile_pool(name="xb", bufs=1) as xb_pool, \
             tc.tile_pool(name="w1", bufs=2) as w1_pool, \
             tc.tile_pool(name="hT", bufs=1) as h_pool, \
             tc.tile_pool(name="w2", bufs=2) as w2_pool, \
             tc.tile_pool(name="sg", bufs=3) as sg_pool, \
             tc.tile_pool(name="yo", bufs=3) as yo_pool, \
             tc.tile_pool(name="psu", bufs=3, space="PSUM") as psu_pool, \
             tc.tile_pool(name="psg", bufs=3, space="PSUM") as psg_pool, \
             tc.tile_pool(name="psy", bufs=2, space="PSUM") as psy_pool:

            for _rep in range(reps):
                xts = []
                off = 0
                for ti, (t0, tw) in enumerate(tiles):
                    xt_t = xb_pool.tile([128, CC * tw], BF16, tag=f"xb{ti}")
                    nc.sync.dma_start(xt_t[:], xtb[:, off:off + CC * tw])
                    xts.append(xt_t)
                    off += CC * tw

                # hT = silu(x@wg) * (x@wi), transposed: [H, cap] bf16
                hT = h_pool.tile([128, HH * cap], BF16, tag="hT")

                # ---- stage 1 ----
                for hb in range(HB):
                    wib_t = w1_pool.tile([128, BLK], BF16, tag="wib")
                    nc.sync.dma_start(wib_t[:],
                                      wib[:, hb * BLK:(hb + 1) * BLK])
                    wgb_t = w1_pool.tile([128, BLK], BF16, tag="wgb")
                    nc.sync.dma_start(wgb_t[:],
                                      wgb[:, hb * BLK:(hb + 1) * BLK])
                    for hi in range(HBW // 128):
                        hh = hb * (HBW // 128) + hi
                        for ti, (t0, tw) in enumerate(tiles):
                            ps_u = psu_pool.tile([128, 512], F32, tag="psu")
                            ps_g = psg_pool.tile([128, 512], F32, tag="psg")
                            # interleaved u/g chains: consecutive matmuls
                            # always target different PSUM banks
                            for cc in range(CC):
                                nc.tensor.matmul(
                                    ps_u[:, :tw],
                                    wib_t[:, cc * HBW + hi * 128:
                                          cc * HBW + (hi + 1) * 128],
                                    xts[ti][:, cc * tw:(cc + 1) * tw],
                                    start=(cc == 0), stop=(cc == CC - 1))
                                nc.tensor.matmul(
                                    ps_g[:, :tw],
                                    wgb_t[:, cc * HBW + hi * 128:
                                          cc * HBW + (hi + 1) * 128],
                                    xts[ti][:, cc * tw:(cc + 1) * tw],
                                    start=(cc == 0), stop=(cc == CC - 1))
                            sg = sg_pool.tile([128, 512], F32, tag="sg")
                            nc.scalar.activation(
                                sg[:, :tw], ps_g[:, :tw],
                                mybir.ActivationFunctionType.Silu)
                            nc.vector.tensor_mul(
                                hT[:, hh * cap + t0: hh * cap + t0 + tw],
                                ps_u[:, :tw], sg[:, :tw])

                # ---- stage 2: yT = wo^T @ hT ----
                for cb in range(CB):
                    wob_t = w2_pool.tile([128, BLK2], BF16, tag="wob")
                    nc.sync.dma_start(wob_t[:],
                                      wob[:, cb * BLK2:(cb + 1) * BLK2])
                    for ci in range(CBW // 128):
                        c0 = cb * CBW + ci * 128
                        for ti, (t0, tw) in enumerate(tiles):
                            ps_y = psy_pool.tile([128, 512], F32, tag="psy")
                            for hh in range(HH):
                                nc.tensor.matmul(
                                    ps_y[:, :tw],
                                    wob_t[:, hh * CBW + ci * 128:
                                          hh * CBW + (ci + 1) * 128],
                                    hT[:, hh * cap + t0: hh * cap + t0 + tw],
                                    start=(hh == 0), stop=(hh == HH - 1))
                            yo = yo_pool.tile([128, 512], BF16, tag="yo")
                            nc.vector.tensor_copy(yo[:, :tw], ps_y[:, :tw])
                            nc.sync.dma_start(yt[c0:c0 + 128, t0:t0 + tw],
                                              yo[:, :tw])
                            if cb == CB - 1 and ci == CBW // 128 - 1 \
                                    and ti == len(tiles) - 1:
                                nc.sync.dma_start(done[0:1, 0:8], yo[0:1, 0:8])
    _split_multi_waits(nc)
    return nc


def pack_wi(w):
    """wi/wg [C, H] f32 -> [128, HB*CC*HBW] bf16 in the wib DRAM layout."""
    a = np.asarray(w).reshape(CC, 128, HB, HBW)          # [cc, p, hb, f]
    a = a.transpose(1, 2, 0, 3)                          # [p, hb, cc, f]
    return np.ascontiguousarray(a.reshape(128, HB * CC * HBW)
                                ).astype(ml_dtypes.bfloat16)


def pack_wo(w):
    """wo [H, C] f32 -> [128, CB*HH*CBW] bf16 in the wob DRAM layout."""
    a = np.asarray(w).reshape(HH, 128, CB, CBW)          # [hh, p, cb, f]
    a = a.transpose(1, 2, 0, 3)                          # [p, cb, hh, f]
    return np.ascontiguousarray(a.reshape(128, CB * HH * CBW)
                                ).astype(ml_dtypes.bfloat16)


def pack_x(x_disp_T, tiles):
    """x^T dispatch slab [C, cap] f32 -> [128, CC*cap] bf16, tile-major
    per the given (t0, tw) tile list."""
    a = x_disp_T.reshape(CC, 128, x_disp_T.shape[1])        # [cc, p, t]
    parts = []
    for t0, tw in tiles:
        blk = a[:, :, t0:t0 + tw].transpose(1, 0, 2)        # [p, cc, tw]
        parts.append(blk.reshape(128, CC * tw))
    return np.ascontiguousarray(np.concatenate(parts, axis=1)
                                ).astype(ml_dtypes.bfloat16)


def _route(x, w_gate):
    """Host-side router. Runs the exact reference ops on jax-CPU so the
    top-2 selection and gate values match the reference bit-for-bit."""
    import jax
    import jax.numpy as jnp
    cpu = jax.devices("cpu")[0]
    with jax.default_device(cpu):
        xj = jnp.asarray(np.asarray(x))
        wj = jnp.asarray(np.asarray(w_gate))
        logits = jnp.einsum("btc,ce->bte", xj, wj)
        gates = jax.nn.softmax(logits, axis=-1)
        topk_vals, topk_idx = jax.lax.top_k(gates, TOP_K)
    return (np.asarray(topk_vals).reshape(-1, TOP_K),
            np.asarray(topk_idx).reshape(-1, TOP_K))


def _dispatch(x, topk_idx):
    """Token lists per expert, (token, slot) positions, cap, and the
    gathered+packed per-expert xtb slabs."""
    N = x.shape[0] * x.shape[1] if x.ndim == 3 else x.shape[0]
    x_flat = np.ascontiguousarray(np.asarray(x).reshape(N, C))
    idx_lists = []
    pos = np.empty((N, TOP_K), dtype=np.int64)
    for e in range(N_EXPERTS):
        sel = (topk_idx == e)
        toks = np.flatnonzero(sel.any(axis=1))
        idx_lists.append(toks)
        pos_of = np.full(N, -1, dtype=np.int64)
        pos_of[toks] = np.arange(len(toks))
        for k in range(TOP_K):
            m = sel[:, k]
            pos[m, k] = pos_of[m]
    max_cnt = max(len(t) for t in idx_lists)
    cap = max(512, -(-max_cnt // 4) * 4)
    tiles = tok_tiles_for(cap)

    xT = np.ascontiguousarray(x_flat.T)            # [C, N]
    xtbs = []
    for e in range(N_EXPERTS):
        toks = idx_lists[e]
        slab = np.zeros((C, cap), dtype=np.float32)
        slab[:, :len(toks)] = xT[:, toks]
        xtbs.append(pack_x(slab, tiles))
    return idx_lists, pos, cap, xtbs


def make_in_maps(x, wi, wg, wo, topk_idx):
    idx_lists, pos, cap, xtbs = _dispatch(x, topk_idx)
    in_maps = []
    for e in range(N_EXPERTS):
        in_maps.append({
            "xtb": xtbs[e],
            "wib": pack_wi(wi[e]),
            "wgb": pack_wi(wg[e]),
            "wob": pack_wo(wo[e]),
        })
    return idx_lists, pos, cap, in_maps


def kernel(x, w_gate, wi, wg, wo):
    x = np.asarray(x)
    wi, wg, wo = np.asarray(wi), np.asarray(wg), np.asarray(wo)
    N = B * T

    topk_vals, topk_idx = _route(x, w_gate)
    idx_lists, pos, cap, in_maps = make_in_maps(x, wi, wg, wo, topk_idx)

    nc = build_program(cap)
    res = run_bass_kernel_spmd(nc, in_maps, core_ids=list(range(N_CORES)))

    # combine: out[t] = sum_k vals[t,k] * y_{idx[t,k]}[t]
    Y = np.empty((N_EXPERTS, cap, C), dtype=np.float32)   # token-major
    for e in range(N_EXPERTS):
        Y[e] = res.results[e]["yt"].astype(np.float32).T
    out = (topk_vals[:, 0:1] * Y[topk_idx[:, 0], pos[:, 0], :]
           + topk_vals[:, 1:2] * Y[topk_idx[:, 1], pos[:, 1], :])
    return out.reshape(B, T, C).astype(np.float32)
